# revision 4
# baseline (speedup 1.0000x reference)
"""Trainium2 Bass kernel for nn_ConvAttention (sparse_attention).

Reference computes, per batch b and query position i (along L):
    qkv = W1 @ x (1x1 conv);  Q,K,V split
    S[b,i,j] = conv5x5(Q[b,i] + K[b,j]) + b2
    attn     = softmax_j(S)
    out[b,i] = sum_j attn[b,i,j] * V[b,j]

Key algebra (exact, not approximate):
  * conv is linear => conv(Q_i + K_j) = conv(Q_i) + conv(K_j).
  * conv(Q_i), b2, and the conv of K's bias b1k are all constant along the
    softmax axis j, so they cancel in softmax exactly.
  * Therefore attn is independent of i, and out[b,:, :, :, i] is the same for
    every i:  out = sum_j softmax_j(conv(W1k @ x_j)) * (W1v @ x_j + b1v).
  * The 1x1 K-projection folds into the 5x5 conv weights on the host:
    W2eff[o,c,dy,dx] = sum_k W2[o,k,dy,dx] * W1k[k,c].

Device work per core (H sharded, 2 output rows + 2-row halo per core):
  * Score conv: 25 taps x 2 row-banks of matmuls (K=128 via block-diagonal
    batch packing: partitions = b*64+c), accumulated in PSUM. The
    block-diagonal weights are built on device from a single 64-wide block
    (halves the weight DMA traffic).
  * V projection reuses the conv's input rows (no separate DMA).
  * softmax over l (innermost, 32 contiguous) + weighted V sum on ACT/DVE.
Host: pad/shard x, fold weights, gather 16KB/core outputs, add b1v,
broadcast over l.
"""

import os

import numpy as np

B, C, H, W, L = 2, 64, 16, 16, 32
NCORES = 8
RPC = H // NCORES          # output rows per core (2)
HALO = RPC + 4             # input rows held per core (6)
WPAD = W + 4               # zero-padded width (20)
P = 2 * C                  # partitions: b*64 + channel

CONV_DT = os.environ.get("CONV_DT", "f32r")   # f32 | f32r | bf16
N_WARMUP = int(os.environ.get("N_WARMUP", "7"))

_PLAN = None


def _mm_dt(mybir, name):
    return {"f32": mybir.dt.float32,
            "f32r": mybir.dt.float32r,
            "bf16": mybir.dt.bfloat16}[name]


def _np_dt(name):
    if name == "bf16":
        import ml_dtypes
        return ml_dtypes.bfloat16
    return np.float32


class _Plan:
    def __init__(self):
        import concourse.bacc as bacc
        import concourse.tile as tile
        from concourse import mybir

        f32 = mybir.dt.float32
        cdt = _mm_dt(mybir, CONV_DT)
        bf16 = mybir.dt.bfloat16
        nc = bacc.Bacc("TRN2", target_bir_lowering=False, debug=False,
                       num_devices=NCORES)

        xh_d = nc.dram_tensor("xh", [P, HALO, WPAD, L], cdt, kind="ExternalInput")
        w2_d = nc.dram_tensor("w2", [C, 25, C], cdt, kind="ExternalInput")
        wv_d = nc.dram_tensor("wv", [P, P], cdt, kind="ExternalInput")
        o_d = nc.dram_tensor("o", [P, RPC, W], f32, kind="ExternalOutput")

        taps = [(dy, dx) for dy in range(-2, 3) for dx in range(-2, 3)]

        with tile.TileContext(nc) as tc:
            with (
                tc.tile_pool(name="sb", bufs=1) as sb,
                tc.tile_pool(name="work", bufs=2) as work,
                tc.tile_pool(name="psum", bufs=2, space="PSUM") as psum,
            ):
                # PE warmup during input DMA: keeps the ramp (HAM) going so
                # the conv starts at full clock. No DMA deps.
                if N_WARMUP:
                    wdum = sb.tile([P, P], bf16, tag="wdum")
                    rdum = sb.tile([P, 512], bf16, tag="rdum")
                    nc.gpsimd.memset(wdum[:], 0)
                    nc.gpsimd.memset(rdum[:], 0)
                    pdum = psum.tile([P, 512], f32, tag="pdum")
                    for i in range(N_WARMUP):
                        nc.tensor.matmul(pdum[:], lhsT=wdum[:], rhs=rdum[:],
                                         start=(i == 0), stop=(i == N_WARMUP - 1))

                # Block-diagonal conv weights built on device:
                # w2sb[0:64, t, 0:64] = w2sb[64:128, t, 64:128] = eff[c, t, o]
                w2sb = sb.tile([P, 25, P], cdt, tag="w2sb")
                # memset rejects float32r; same bits via a float32 view
                w2sb_v = w2sb[:] if cdt != mybir.dt.float32r else \
                    w2sb[:].bitcast(mybir.dt.float32)
                nc.gpsimd.memset(w2sb_v, 0)
                nc.sync.dma_start(out=w2sb[0:C, :, 0:C], in_=w2_d[:])
                nc.sync.dma_start(out=w2sb[C:P, :, C:P], in_=w2sb[0:C, :, 0:C])

                # input DMAs, in the order the conv consumes them
                xrow = []
                for i in range(HALO):
                    t = sb.tile([P, WPAD, L], cdt, tag=f"xh{i}")
                    nc.sync.dma_start(out=t[:], in_=xh_d[:, i])
                    xrow.append(t)
                wv_t = sb.tile([P, P], cdt, tag="wv")
                nc.sync.dma_start(out=wv_t[:], in_=wv_d[:])

                # score conv: accumulate 25 taps into one PSUM bank per row
                cks = []
                for r in range(RPC):
                    ck = psum.tile([P, W * L], f32, tag="ck")
                    for ti, (dy, dx) in enumerate(taps):
                        nc.tensor.matmul(
                            ck[:],
                            lhsT=w2sb[:, ti, :],
                            rhs=xrow[2 + r + dy][:, 2 + dx:2 + dx + W, :],
                            start=(ti == 0),
                            stop=(ti == len(taps) - 1),
                        )
                    cks.append(ck)

                # V projection reuses the conv input rows (interior columns)
                vps = []
                for r in range(RPC):
                    vp = psum.tile([P, W * L], f32, tag="vp")
                    nc.tensor.matmul(vp[:], lhsT=wv_t[:],
                                     rhs=xrow[2 + r][:, 2:2 + W, :],
                                     start=True, stop=True)
                    vps.append(vp)

                # softmax over l + weighted V sum; single output tile
                o_t = sb.tile([P, RPC, W], f32, tag="o")
                for r in range(RPC):
                    e = work.tile([P, W, L], f32, tag="e")
                    nc.scalar.activation(e[:], cks[r][:],
                                         func=mybir.ActivationFunctionType.Exp)
                    s = work.tile([P, W], f32, tag="s")
                    nc.vector.tensor_reduce(out=s[:], in_=e[:],
                                            axis=mybir.AxisListType.X,
                                            op=mybir.AluOpType.add)
                    rcp = work.tile([P, W], f32, tag="rcp")
                    nc.vector.reciprocal(rcp[:], s[:])
                    tt = work.tile([P, W, L], f32, tag="tt")
                    nc.vector.tensor_mul(tt[:], e[:],
                                         vps[r][:].rearrange("p (w l) -> p w l", l=L))
                    u = work.tile([P, W], f32, tag="u")
                    nc.vector.tensor_reduce(out=u[:], in_=tt[:],
                                            axis=mybir.AxisListType.X,
                                            op=mybir.AluOpType.add)
                    nc.vector.tensor_mul(o_t[:, r], u[:], rcp[:])
                nc.sync.dma_start(out=o_d[:], in_=o_t[:])

        nc.compile()
        self.nc = nc


def _get_plan():
    global _PLAN
    if _PLAN is None:
        _PLAN = _Plan()
    return _PLAN


def _prep_in_maps(x, W1, W2):
    cnp = _np_dt(CONV_DT)

    # Fold the K-projection into the conv weights (in float64 for accuracy).
    W1k = W1[C:2 * C, :, 0, 0].astype(np.float64)          # [k, c]
    W2eff = np.einsum("okyx,kc->ocyx", W2.astype(np.float64), W1k)
    eff = np.ascontiguousarray(
        W2eff.transpose(1, 2, 3, 0).reshape(C, 25, C)      # [c_in, tap, o]
    ).astype(np.float32).astype(cnp)

    W1v = W1[2 * C:3 * C, :, 0, 0]                          # [o, c]
    wvp = np.zeros((P, P), np.float32)
    wvp[:C, :C] = W1v.T
    wvp[C:, C:] = W1v.T
    wvp = wvp.astype(cnp)

    in_maps = []
    for m in range(NCORES):
        g0 = RPC * m - 2
        buf = np.zeros((B, C, HALO, WPAD, L), np.float32)
        lo, hi = max(g0, 0), min(g0 + HALO, H)
        buf[:, :, lo - g0:hi - g0, 2:2 + W, :] = x[:, :, lo:hi, :, :]
        xh = buf.reshape(P, HALO, WPAD, L).astype(cnp)
        in_maps.append({"xh": xh, "w2": eff, "wv": wvp})
    return in_maps


def kernel(x, W1, b1, W2, b2):
    from concourse.bass_utils import run_bass_kernel_spmd

    x = np.asarray(x, dtype=np.float32)
    W1 = np.asarray(W1, dtype=np.float32)
    b1 = np.asarray(b1, dtype=np.float32)
    W2 = np.asarray(W2, dtype=np.float32)

    plan = _get_plan()
    in_maps = _prep_in_maps(x, W1, W2)
    res = run_bass_kernel_spmd(plan.nc, in_maps, core_ids=list(range(NCORES)))

    b1v = b1[2 * C:3 * C].astype(np.float32)
    out = np.empty((B, C, H, W, L), np.float32)
    for m in range(NCORES):
        o = res.results[m]["o"].reshape(B, C, RPC, W)       # [b, c, r, w]
        o = o + b1v[None, :, None, None]
        out[:, :, RPC * m:RPC * (m + 1), :, :] = o[..., None]
    return out


# revision 7
# speedup vs baseline: 2.3327x; 2.3327x over previous
"""Trainium2 Bass kernel for nn_ConvAttention (sparse_attention).

Reference computes, per batch b and query position i (along L):
    qkv = W1 @ x (1x1 conv);  Q,K,V split
    S[b,i,j] = conv5x5(Q[b,i] + K[b,j]) + b2
    attn     = softmax_j(S)
    out[b,i] = sum_j attn[b,i,j] * V[b,j]

Key algebra (exact, not approximate):
  * conv is linear => conv(Q_i + K_j) = conv(Q_i) + conv(K_j).
  * conv(Q_i), b2, and the conv of K's bias b1k are all constant along the
    softmax axis j, so they cancel in softmax exactly.
  * Therefore attn is independent of i, and out[b,:, :, :, i] is the same for
    every i:  out = sum_j softmax_j(conv(W1k @ x_j)) * (W1v @ x_j + b1v).
  * The 1x1 K-projection folds into the 5x5 conv weights on the host:
    W2eff[o,c,dy,dx] = sum_k W2[o,k,dy,dx] * W1k[k,c].

Device work per core (H sharded, 2 output rows + 2-row halo per core):
  * Score conv: 25 taps x 2 row-banks of matmuls (K=128 via block-diagonal
    batch packing: partitions = b*64+c), accumulated in PSUM. Taps are
    consumed chunk-by-chunk (5 taps per weight-DMA chunk) interleaved across
    both row banks so compute starts as soon as the first chunk lands.
  * V projection reuses the conv's input rows (no separate DMA).
  * softmax over l (innermost, 32 contiguous): exp on ACT, sums on DVE,
    E*V on GPSIMD to keep it off the serial DVE tail.
Host: pad/shard x, fold weights, gather 16KB/core outputs, add b1v,
broadcast over l.
"""

import os

import numpy as np

B, C, H, W, L = 2, 64, 16, 16, 32
NCORES = 8
RPC = H // NCORES          # output rows per core (2)
HALO = RPC + 4             # input rows held per core (6)
WPAD = W + 4               # zero-padded width (20)
P = 2 * C                  # partitions: b*64 + channel

CONV_DT = os.environ.get("CONV_DT", "f32r")   # f32 | f32r | bf16
N_WARMUP = int(os.environ.get("N_WARMUP", "7" if CONV_DT != "bf16" else "4"))

_PLAN = None


def _mm_dt(mybir, name):
    return {"f32": mybir.dt.float32,
            "f32r": mybir.dt.float32r,
            "bf16": mybir.dt.bfloat16}[name]


def _np_dt(name):
    if name == "bf16":
        import ml_dtypes
        return ml_dtypes.bfloat16
    return np.float32


class _Plan:
    def __init__(self):
        import concourse.bacc as bacc
        import concourse.tile as tile
        from concourse import mybir

        f32 = mybir.dt.float32
        cdt = _mm_dt(mybir, CONV_DT)
        bf16 = mybir.dt.bfloat16
        nc = bacc.Bacc("TRN2", target_bir_lowering=False, debug=False,
                       num_devices=NCORES)

        xh_d = nc.dram_tensor("xh", [P, HALO, WPAD, L], cdt, kind="ExternalInput")
        w2_d = nc.dram_tensor("w2", [P, 25, P], cdt, kind="ExternalInput")
        wv_d = nc.dram_tensor("wv", [P, P], cdt, kind="ExternalInput")
        o_d = nc.dram_tensor("o", [P, RPC, W], f32, kind="ExternalOutput")

        with tile.TileContext(nc) as tc:
            with (
                tc.tile_pool(name="sb", bufs=1) as sb,
                tc.tile_pool(name="work", bufs=2) as work,
                tc.tile_pool(name="psum", bufs=2, space="PSUM") as psum,
            ):
                # PE warmup during input DMA: keeps the clock ramp going so
                # the conv starts at full speed. No DMA deps.
                if N_WARMUP:
                    wdum = sb.tile([P, P], bf16, tag="wdum")
                    rdum = sb.tile([P, 512], bf16, tag="rdum")
                    nc.gpsimd.memset(wdum[:], 0)
                    nc.gpsimd.memset(rdum[:], 0)
                    pdum = psum.tile([P, 512], f32, tag="pdum")
                    for i in range(N_WARMUP):
                        nc.tensor.matmul(pdum[:], lhsT=wdum[:], rhs=rdum[:],
                                         start=(i == 0), stop=(i == N_WARMUP - 1))

                # Input DMAs interleaved in consumption order: the conv eats
                # (xh row, w2 chunk) pairs; chunk c needs rows c..c+1.
                xrow = [sb.tile([P, WPAD, L], cdt, tag=f"xh{i}", name=f"xh{i}")
                        for i in range(HALO)]
                w2c = [sb.tile([P, 5, P], cdt, tag=f"w2{i}", name=f"w2{i}")
                       for i in range(5)]
                nc.sync.dma_start(out=xrow[0][:], in_=xh_d[:, 0])
                nc.sync.dma_start(out=xrow[1][:], in_=xh_d[:, 1])
                for c in range(5):
                    nc.sync.dma_start(out=w2c[c][:], in_=w2_d[:, 5 * c:5 * (c + 1), :])
                    if c + 2 < HALO:
                        nc.sync.dma_start(out=xrow[c + 2][:], in_=xh_d[:, c + 2])
                wv_t = sb.tile([P, P], cdt, tag="wv")
                nc.sync.dma_start(out=wv_t[:], in_=wv_d[:])

                # Score conv: accumulate 25 taps into one PSUM bank per output
                # row r. Chunk c holds taps with dy = c-2, reading xh row r+c.
                cks = [psum.tile([P, W * L], f32, tag="ck", name=f"ck{r}")
                       for r in range(RPC)]
                vps, v_s = [], []

                def chunk(c, r):
                    for k in range(5):
                        ti = 5 * c + k
                        nc.tensor.matmul(
                            cks[r][:],
                            lhsT=w2c[c][:, k, :],
                            rhs=xrow[r + c][:, k:k + W, :],
                            start=(ti == 0),
                            stop=(ti == 24),
                        )

                for c in range(4):
                    chunk(c, 0)
                    chunk(c, 1)
                    if c == 2:
                        # V projection reuses interior of conv input rows;
                        # ACT copies V out of PSUM while the conv continues.
                        for r in range(RPC):
                            vp = psum.tile([P, W * L], f32, tag="vp")
                            nc.tensor.matmul(vp[:], lhsT=wv_t[:],
                                             rhs=xrow[2 + r][:, 2:2 + W, :],
                                             start=True, stop=True)
                            vs = sb.tile([P, W, L], f32, tag=f"v{r}")
                            nc.scalar.copy(vs[:], vp[:])
                            vps.append(vp)
                            v_s.append(vs)
                chunk(4, 0)
                chunk(4, 1)

                # softmax over l + weighted V sum.
                for r in range(RPC):
                    e = work.tile([P, W, L], f32, tag="e")
                    nc.scalar.activation(e[:], cks[r][:],
                                         func=mybir.ActivationFunctionType.Exp)
                    s = work.tile([P, W], f32, tag="s")
                    nc.vector.tensor_reduce(out=s[:], in_=e[:],
                                            axis=mybir.AxisListType.X,
                                            op=mybir.AluOpType.add)
                    rcp = work.tile([P, W], f32, tag="rcp")
                    nc.vector.reciprocal(rcp[:], s[:])
                    tt = work.tile([P, W, L], f32, tag="tt")
                    nc.gpsimd.tensor_mul(tt[:], e[:], v_s[r][:])
                    u = work.tile([P, W], f32, tag="u")
                    nc.vector.tensor_reduce(out=u[:], in_=tt[:],
                                            axis=mybir.AxisListType.X,
                                            op=mybir.AluOpType.add)
                    o_t = work.tile([P, W], f32, tag="o")
                    nc.vector.tensor_mul(o_t[:], u[:], rcp[:])
                    nc.sync.dma_start(out=o_d[:, r], in_=o_t[:])

        nc.compile()
        self.nc = nc


def _get_plan():
    global _PLAN
    if _PLAN is None:
        _PLAN = _Plan()
    return _PLAN


def _prep_in_maps(x, W1, W2):
    cnp = _np_dt(CONV_DT)

    # Fold the K-projection into the conv weights (in float64 for accuracy).
    W1k = W1[C:2 * C, :, 0, 0].astype(np.float64)          # [k, c]
    W2eff = np.einsum("okyx,kc->ocyx", W2.astype(np.float64), W1k)
    eff = np.ascontiguousarray(
        W2eff.transpose(1, 2, 3, 0).reshape(C, 25, C)      # [c_in, tap, o]
    ).astype(np.float32)
    w2p = np.zeros((P, 25, P), np.float32)
    w2p[:C, :, :C] = eff
    w2p[C:, :, C:] = eff
    w2p = w2p.astype(cnp)

    W1v = W1[2 * C:3 * C, :, 0, 0]                          # [o, c]
    wvp = np.zeros((P, P), np.float32)
    wvp[:C, :C] = W1v.T
    wvp[C:, C:] = W1v.T
    wvp = wvp.astype(cnp)

    in_maps = []
    for m in range(NCORES):
        g0 = RPC * m - 2
        buf = np.zeros((B, C, HALO, WPAD, L), np.float32)
        lo, hi = max(g0, 0), min(g0 + HALO, H)
        buf[:, :, lo - g0:hi - g0, 2:2 + W, :] = x[:, :, lo:hi, :, :]
        xh = buf.reshape(P, HALO, WPAD, L).astype(cnp)
        in_maps.append({"xh": xh, "w2": w2p, "wv": wvp})
    return in_maps


def kernel(x, W1, b1, W2, b2):
    from concourse.bass_utils import run_bass_kernel_spmd

    x = np.asarray(x, dtype=np.float32)
    W1 = np.asarray(W1, dtype=np.float32)
    b1 = np.asarray(b1, dtype=np.float32)
    W2 = np.asarray(W2, dtype=np.float32)

    plan = _get_plan()
    in_maps = _prep_in_maps(x, W1, W2)
    res = run_bass_kernel_spmd(plan.nc, in_maps, core_ids=list(range(NCORES)))

    b1v = b1[2 * C:3 * C].astype(np.float32)
    out = np.empty((B, C, H, W, L), np.float32)
    for m in range(NCORES):
        o = res.results[m]["o"].reshape(B, C, RPC, W)       # [b, c, r, w]
        o = o + b1v[None, :, None, None]
        out[:, :, RPC * m:RPC * (m + 1), :, :] = o[..., None]
    return out
